# revision 11
# baseline (speedup 1.0000x reference)
"""Trainium2 Bass kernel for nn_MultiHeadAttention_35321811043105.

Module semantics (faithful to the reference's reshape): per batch b and
"head" h, the head covers sequence rows [128h, 128h+128); its [128, 1024]
activation block is reinterpreted row-major as a [2048, 64] matrix.
Attention (scores/softmax/attn@V) runs on those [2048, 64] matrices, and the
[2048, 64] result is reinterpreted back to [128, 1024] before the output
projection.

Sharding: 32 (b, h) pairs across 8 cores, 4 pairs per core (data parallel,
batch*heads). Each core computes its pairs' attn [2048, 2048] and proj
[128, 1024] outputs; the host reassembles.

Host-side prep (layout only, no FLOPs): weights passed transposed (W.T),
per-pair activation blocks passed transposed (X.T), biases packed.
"""

import os

import numpy as np

import concourse.bass as bass
import concourse.bacc as bacc
import concourse.mybir as mybir
import concourse.tile as tile
from concourse import bass_utils
from concourse.masks import make_identity

F32 = mybir.dt.float32
AF = mybir.ActivationFunctionType

B, S, D = 2, 2048, 1024
H = 16
NCORES = 8
PAIRS = 4          # (b, h) pairs per core
BLK = 128          # sequence rows per head block
NQ = 2048          # rows of the reshaped per-head matrix
DD = 64            # cols of the reshaped per-head matrix
SCALE = 1.0 / 8.0  # 1/sqrt(DD)

_CACHE = {}


def _emit(nc, tc, io):
    ones = io["ones"]
    ident = io["ident"]
    bias_sb_dram = io["bias_pack"]

    from contextlib import ExitStack
    stack = ExitStack()
    pool_w = stack.enter_context(tc.tile_pool(name="w", bufs=1))
    pool_stage = stack.enter_context(tc.tile_pool(name="stage", bufs=2))
    pool_qtt = stack.enter_context(tc.tile_pool(name="qtt", bufs=PAIRS))
    pool_ktt = stack.enter_context(tc.tile_pool(name="ktt", bufs=PAIRS))
    pool_v2k = stack.enter_context(tc.tile_pool(name="v2k", bufs=PAIRS))
    pool_attn = stack.enter_context(tc.tile_pool(name="attn", bufs=2))
    pool_est = stack.enter_context(tc.tile_pool(name="est", bufs=2))
    pool_outt = stack.enter_context(tc.tile_pool(name="outt", bufs=1))
    pool_psb = stack.enter_context(tc.tile_pool(name="psb", bufs=2))
    pool_small = stack.enter_context(tc.tile_pool(name="small", bufs=2))
    pool_strip = stack.enter_context(
        tc.tile_pool(name="strip", bufs=2, space="PSUM"))
    pool_acc = stack.enter_context(
        tc.tile_pool(name="acc", bufs=2, space="PSUM"))
    pool_pp = stack.enter_context(
        tc.tile_pool(name="pp", bufs=2, space="PSUM"))
    pool_dram = stack.enter_context(
        tc.tile_pool(name="vscr", bufs=2, space="DRAM"))

    qtt = [None] * PAIRS   # [128, 2048] Q2048^T, rows 0-63 = dd, 64-127 dup
    ktt = [None] * PAIRS
    v2k = [None] * PAIRS   # [128, 16, 65] V2048 tiles + ones column

    # ---- Phase A: projections, per weight tensor across all pairs ----
    for tix, (xt_dram, wt_dram) in enumerate(
        [(io["xq_t"], io["wq_t"]), (io["xk_t"], io["wk_t"]),
         (io["xv_t"], io["wv_t"])]
    ):
        w_sb = pool_w.tile([128, 8, 1024], F32, tag="w")
        nc.sync.dma_start(
            out=w_sb, in_=wt_dram.rearrange("(c p) o -> p c o", p=128))
        bb = pool_stage.tile([128, 1024], F32, tag="bb")
        nc.gpsimd.dma_start(
            out=bb, in_=bass.AP(tensor=bias_sb_dram.tensor, offset=1024 * tix,
                                ap=[[0, 128], [1, 1024]]))
        for p in range(PAIRS):
            xt = pool_stage.tile([128, 8, 128], F32, tag="stage")
            nc.sync.dma_start(
                out=xt, in_=xt_dram[p].rearrange("(c p) i -> p c i", p=128))
            nat = pool_stage.tile([128, 1024], F32, tag="stage")
            for oc in range(2):
                pp = pool_pp.tile([128, 512], F32, tag="pp")
                for fc in range(8):
                    nc.tensor.matmul(
                        pp, xt[:, fc, :], w_sb[:, fc, 512 * oc:512 * (oc + 1)],
                        start=(fc == 0), stop=(fc == 7))
                nc.vector.tensor_add(nat[:, 512 * oc:512 * (oc + 1)],
                                     pp, bb[:, 512 * oc:512 * (oc + 1)])
            if tix < 2:
                # Q/K: 16 PE transposes [128, 64] -> [64, 128], then DVE
                # shuffle-copies into class->consecutive-q column order.
                tt = (pool_qtt if tix == 0 else pool_ktt).tile(
                    [128, 2048], F32, tag="qtt" if tix == 0 else "ktt")
                ttr = tt.rearrange("p (i j) -> p j i", j=16)
                for g in range(4):
                    pp = pool_pp.tile([128, 512], F32, tag="pp")
                    for t4 in range(4):
                        j = 4 * g + t4
                        nc.tensor.matmul(
                            pp[0:64, 128 * t4:128 * (t4 + 1)],
                            nat[:, 64 * j:64 * (j + 1)], ident,
                            is_transpose=True,
                            start=(t4 == 0), stop=(t4 == 3))
                    nc.vector.tensor_copy(
                        out=ttr[0:64, 4 * g:4 * (g + 1), :],
                        in_=pp[0:64, :].rearrange("p (j i) -> p j i", i=128))
                nc.sync.dma_start(out=tt[64:128, :], in_=tt[0:64, :])
                if tix == 0:
                    qtt[p] = tt
                else:
                    ktt[p] = tt
            else:
                # V: round-trip through DRAM to realize the [2048, 64] view.
                vs = pool_dram.tile([128, 1024], F32, tag="vscr")
                nc.sync.dma_start(out=vs, in_=nat)
                vt = pool_v2k.tile([128, 16, 65], F32, tag="v2k")
                nc.vector.memset(vt[:, :, 64:65], 1.0)
                nc.sync.dma_start(
                    out=vt[:, :, 0:64],
                    in_=vs.rearrange("a b -> (a b)").rearrange(
                        "(kc pp d) -> pp kc d", pp=128, d=64))
                v2k[p] = vt

    wo_sb = pool_w.tile([128, 16, 512], F32, tag="w")
    nc.sync.dma_start(out=wo_sb, in_=io["wo_t"])
    bbo = pool_stage.tile([128, 1024], F32, tag="bb")
    nc.gpsimd.dma_start(
        out=bbo, in_=bass.AP(tensor=bias_sb_dram.tensor, offset=3072,
                             ap=[[0, 128], [1, 1024]]))

    phases = os.environ.get("KPH", "BCD")

    # ---- Phases B/C/D per pair ----
    for p in range(PAIRS):
        qt, kt, vt = qtt[p], ktt[p], v2k[p]

        if "A1" in phases:
            # debug: dump A products into attn_out rows
            nc.sync.dma_start(out=io["attn_out"][p, 0:128, 0:2048], in_=qt)
            nc.sync.dma_start(out=io["attn_out"][p, 128:256, 0:2048], in_=kt)
            nc.sync.dma_start(
                out=io["attn_out"][p, 256:384, 0:1040],
                in_=vt.rearrange("a b c -> a (b c)"))
            continue

        # B: scores in q-layout, exp+rowsum, normalize, DMA attn out.
        for c in range(16 if "B" in phases else 0):
            base = 64 * (c % 2)
            lhs = qt[base:base + 64, 128 * c:128 * (c + 1)]
            sums = pool_small.tile([128, 2], F32, tag="sums")
            strips = []
            for kk in range(2):
                st = pool_strip.tile([128, 1024], F32, tag="strip")
                for k2 in range(2):
                    nc.tensor.matmul(
                        st[:, 512 * k2:512 * (k2 + 1)], lhs,
                        kt[base:base + 64,
                           1024 * kk + 512 * k2:1024 * kk + 512 * (k2 + 1)],
                        start=True, stop=True)
                at = pool_attn.tile([128, 1024], F32, tag="attn")
                nc.scalar.activation(
                    out=at, in_=st, func=AF.Exp, scale=SCALE,
                    accum_out=sums[:, kk:kk + 1])
                strips.append(at)
            rs = pool_small.tile([128, 1], F32, tag="rs")
            nc.vector.reduce_sum(out=rs, in_=sums, axis=mybir.AxisListType.X)
            recip = pool_small.tile([128, 1], F32, tag="recip")
            nc.vector.reciprocal(out=recip, in_=rs)
            for kk in range(2):
                nc.vector.tensor_scalar_mul(strips[kk], strips[kk], recip)
                nc.sync.dma_start(
                    out=io["attn_out"][p, 128 * c:128 * (c + 1),
                                       1024 * kk:1024 * (kk + 1)],
                    in_=strips[kk])

        # C: scores transposed, exp, attn@V (with fused rowsum row).
        if "C" not in phases:
            continue
        out_t = pool_outt.tile([128, 2048], F32, tag="outt")
        for qc in range(4):
            acc = pool_acc.tile([65, 512], F32, tag="acc")
            for kg in range(8):
                st = pool_strip.tile([128, 1024], F32, tag="strip")
                for k2 in range(2):
                    kc = 2 * kg + k2
                    base = 64 * (kc % 2)
                    nc.tensor.matmul(
                        st[:, 512 * k2:512 * (k2 + 1)],
                        kt[base:base + 64, 128 * kc:128 * (kc + 1)],
                        qt[base:base + 64, 512 * qc:512 * (qc + 1)],
                        start=True, stop=True)
                est = pool_est.tile([128, 1024], F32, tag="est")
                nc.scalar.activation(out=est, in_=st, func=AF.Exp, scale=SCALE)
                for k2 in range(2):
                    kc = 2 * kg + k2
                    nc.tensor.matmul(
                        acc, vt[:, kc, :], est[:, 512 * k2:512 * (k2 + 1)],
                        start=(kc == 0), stop=(kc == 15))
            # normalize: out_t[:, qc] = acc[0:64] * (1 / acc[64]) bcast
            recip_t = pool_small.tile([1, 512], F32, tag="sumt")
            nc.vector.tensor_copy(out=recip_t, in_=acc[64:65, :])
            nc.vector.reciprocal(out=recip_t, in_=recip_t)
            ppb = pool_pp.tile([128, 512], F32, tag="pp")
            nc.tensor.matmul(ppb[0:64, :], ones[:, 0:64], recip_t,
                             start=True, stop=True)
            bc = pool_small.tile([64, 512], F32, tag="bc")
            nc.vector.tensor_copy(out=bc, in_=ppb[0:64, :])
            nc.vector.tensor_mul(out_t[0:64, 512 * qc:512 * (qc + 1)],
                                 acc[0:64, :], bc)
        nc.sync.dma_start(out=out_t[64:128, :], in_=out_t[0:64, :])

        # D: output projection proj = outblk @ Wo.T + bo, per j-slab.
        out_tr = out_t.rearrange("p (i j) -> p j i", j=16)
        for oc in range(2 if "D" in phases else 0):
            base = 64 * oc
            pp = pool_pp.tile([128, 512], F32, tag="pp")
            for j in range(16):
                nc.tensor.matmul(
                    pp, out_tr[base:base + 64, j, :],
                    wo_sb[base:base + 64, j, :],
                    start=(j == 0), stop=(j == 15))
            ps = pool_psb.tile([128, 512], F32, tag="psb")
            nc.vector.tensor_add(ps, pp, bbo[:, 512 * oc:512 * (oc + 1)])
            nc.sync.dma_start(
                out=io["proj_out"][p, :, 512 * oc:512 * (oc + 1)], in_=ps)

    stack.close()


def _build():
    if "nc" in _CACHE:
        return _CACHE["nc"]
    nc = bacc.Bacc("TRN2", target_bir_lowering=False, debug=False,
                   num_devices=NCORES)
    io = {}
    for name in ("xq_t", "xk_t", "xv_t"):
        io[name] = nc.dram_tensor(name, [PAIRS, 1024, 128], F32,
                                  kind="ExternalInput").ap()
    for name in ("wq_t", "wk_t", "wv_t"):
        io[name] = nc.dram_tensor(name, [1024, 1024], F32,
                                  kind="ExternalInput").ap()
    io["wo_t"] = nc.dram_tensor("wo_t", [128, 16, 512], F32,
                                kind="ExternalInput").ap()
    io["bias_pack"] = nc.dram_tensor("bias_pack", [1, 4096], F32,
                                     kind="ExternalInput").ap()
    io["attn_out"] = nc.dram_tensor("attn_out", [PAIRS, NQ, NQ], F32,
                                    kind="ExternalOutput").ap()
    io["proj_out"] = nc.dram_tensor("proj_out", [PAIRS, BLK, D], F32,
                                    kind="ExternalOutput").ap()

    with tile.TileContext(nc) as tc:
        with tc.tile_pool(name="singles", bufs=1) as singles:
            ident = singles.tile([128, 128], F32, tag="ident")
            make_identity(nc, ident)
            ones = singles.tile([1, 128], F32, tag="ones")
            nc.vector.memset(ones, 1.0)
            io["ident"] = ident
            io["ones"] = ones
            _emit(nc, tc, io)

    nc.finalize()
    _CACHE["nc"] = nc
    return nc


def kernel(pre_q, pre_k, pre_v, Wq, bq, Wk, bk, Wv, bv, Wo, bo):
    nc = _build()

    pre_q = np.asarray(pre_q, dtype=np.float32)
    pre_k = np.asarray(pre_k, dtype=np.float32)
    pre_v = np.asarray(pre_v, dtype=np.float32)
    wq_t = np.ascontiguousarray(np.asarray(Wq, np.float32).T)
    wk_t = np.ascontiguousarray(np.asarray(Wk, np.float32).T)
    wv_t = np.ascontiguousarray(np.asarray(Wv, np.float32).T)
    wot = np.asarray(Wo, np.float32).T  # [o', o2]
    wo_t = np.empty((128, 16, 512), np.float32)
    for j in range(16):
        wo_t[0:64, j, :] = wot[64 * j:64 * (j + 1), 0:512]
        wo_t[64:128, j, :] = wot[64 * j:64 * (j + 1), 512:1024]
    bias_pack = np.concatenate(
        [np.asarray(x, np.float32).reshape(-1) for x in (bq, bk, bv, bo)]
    ).reshape(1, 4096)

    in_maps = []
    for core in range(NCORES):
        m = {"wq_t": wq_t, "wk_t": wk_t, "wv_t": wv_t, "wo_t": wo_t,
             "bias_pack": bias_pack}
        for name, x in (("xq_t", pre_q), ("xk_t", pre_k), ("xv_t", pre_v)):
            blocks = []
            for p in range(PAIRS):
                gp = core * PAIRS + p
                b, h = gp // H, gp % H
                blocks.append(
                    np.ascontiguousarray(x[b, BLK * h:BLK * (h + 1), :].T))
            m[name] = np.stack(blocks, axis=0)
        in_maps.append(m)

    global _LAST_IN_MAPS
    _LAST_IN_MAPS = in_maps

    res = bass_utils.run_bass_kernel_spmd(
        nc, in_maps, core_ids=list(range(NCORES)))

    attn = np.empty((B, H, NQ, NQ), np.float32)
    proj = np.empty((B, S, D), np.float32)
    for core in range(NCORES):
        for p in range(PAIRS):
            gp = core * PAIRS + p
            b, h = gp // H, gp % H
            attn[b, h] = res.results[core]["attn_out"][p]
            proj[b, BLK * h:BLK * (h + 1), :] = res.results[core]["proj_out"][p]
    return proj, attn
